# revision 4
# baseline (speedup 1.0000x reference)
"""Multi-head self-attention (1x1-conv projections, N=4096 spatial tokens,
C=256 channels, Cq=32) on 8 TRN2 NeuronCores, data-parallel over batch.

Per core (one batch element, x as [C, N]):
  q = wq @ x + bq          [Cq, N]
  k = wk @ x + bk          [Cq, N]
  v = wv @ x               [C, N]   (bv folded into the epilogue)
  S = q^T k                [N, N]
  P = softmax(S, axis=-1)
  out = gamma * (v @ P^T + bv) + x

Layout strategy: compute S^T tiles (keys j on partitions, queries i on the
free dim) so softmax's exp output E^T feeds the PV matmul as the stationary
operand with rhs = [v^T | ones]; the ones column accumulates the softmax
denominator for free (no P transposes, no separate reduction).

fp8 fast path: E is stored as fp8e5 (e5m2) and v as fp8e4 (e4m3) so the PV
matmul runs in DoubleRow perf mode (two key-tiles contracted per
instruction, 2x PE throughput vs bf16). e5m2's ~22-e-folding dynamic range
requires a per-query shift m_i: exp(S_ij - m_i). The shift cancels exactly
in softmax, so m_i only needs to track the row max within ~[-8, +10]; it
is computed ON THE HOST (cheap q/k projections + top-|k|-column and strided
sample maxes + a |q| linear fit) and folded into the energy matmul by
augmenting the contraction dim: q~ = [q; -m; 0...], k~ = [k; 1; 0...]
(K=32 -> 64, which is free on the PE since matmul cost is output-bound).

exp is split between the ACT engine (true exp, fp8e5 output) and the DVE
(Schraudolph bit-trick: bits = round(S~ * 4/ln2 + 59.7) saturating-cast to
uint8, bitcast as e5m2), sized so neither engine bottlenecks. The DVE cast
saturates negatives to 0, which implements exp underflow for free.

dtypes: fp32r (tf32-like, full PE speed at moving-dim>=256) for the
q/k/energy path where exp amplifies absolute error; fp8 for the P*V path
where softmax normalization cancels it.
"""

import numpy as np

import concourse.bass as bass
import concourse.mybir as mybir
import concourse.tile as tile
from concourse.bass_utils import run_bass_kernel_spmd
from concourse.masks import make_identity
from concourse.tile import ScopedClock

F32 = mybir.dt.float32
F32R = mybir.dt.float32r
BF16 = mybir.dt.bfloat16
F8E5 = mybir.dt.float8e5
F8E4 = mybir.dt.float8e4
U8 = mybir.dt.uint8

B, C, CQ = 8, 256, 32
H = W = 64
N = H * W            # 4096 tokens
NCORES = 8
CT = C // 128        # 2 channel tiles
IB = 512             # queries per i-block
N_IB = N // IB       # 8
JT = N // 128        # 32 key tiles
JGRP = 4             # key tiles per exp group (one PSUM S tile pair = 4 banks)
N_JG = JT // JGRP    # 8

# Schraudolph exp-to-e5m2 constants for the DVE share of the softmax
SCHRAU_A = 5.7708017  # 4 / ln 2
SCHRAU_B = 59.7       # 4 * 15 (e5m2 bias) - rounding correction
# which of the 16 S~ PSUM half-tiles per i-block go to the DVE (rest: ACT)
DVE_TILES = frozenset({2, 5, 7, 10, 13, 15})


class PatchedTileContext(tile.TileContext):
    """This walrus build supports only ONE sync-wait command per
    instruction. Peel extra waits into standalone single-wait NOPs on the
    same engine queue, emitted immediately before the instruction (a serial
    conjunction of waits - semantically identical). Same treatment for the
    kernel-tail drain, whose global-clock waits otherwise all land on one
    Drain instruction."""

    MAX_WAITS_PER_INST = 1

    def _add_instruction(self, inst):
        si = inst.sync_info
        waits = list(si.on_wait) if si is not None and si.on_wait else []
        if len(waits) > self.MAX_WAITS_PER_INST and inst.engine is not None:
            keep = waits[-self.MAX_WAITS_PER_INST:]
            peel = waits[: -self.MAX_WAITS_PER_INST]
            for w in peel:
                nop = mybir.InstNoOp(
                    name=self.nc.get_next_instruction_name(),
                    ins=[],
                    outs=[],
                    sync_info=mybir.SyncInfo(on_wait=[w], on_update=[]),
                )
                nop.engine = inst.engine
                super()._add_instruction(nop)
            inst.sync_info = mybir.SyncInfo(
                on_wait=keep,
                on_update=list(si.on_update) if si.on_update else [],
            )
        super()._add_instruction(inst)

    def _drain_and_barrier(self, tick_clock, wait_clock):
        nc = self.nc
        carrier = nc.sync.nop()
        wait_clock.add_sem_waits(
            carrier.ins, ScopedClock({None: tick_clock.global_clock})
        )
        si = carrier.ins.sync_info
        waits = list(si.on_wait) if si is not None and si.on_wait else []
        carrier.ins.sync_info = None
        for w in waits:
            h = bass.SemaphoreHandle(name=w.ant_name or f"sem{w.id}", num=w.id)
            if w.wait_mode == "sem-ge-imm":
                nc.sync.wait_ge(h, w.wait_value)
            else:
                op = {
                    "sem-eq-imm": "eq",
                    "sem-le-imm": "le",
                    "sem-lt-imm": "lt",
                    "sem-gt-imm": "gt",
                }[w.wait_mode]
                nc.sync.wait_op(h, w.wait_value, op)
        nc.sync.drain()
        nc.all_engine_barrier()
        assert self.sems is not None
        popped = nc._tile_sem_poison_stack.pop()
        assert popped is self._sem_poison
        nc.clear_and_free_semaphores(list(self.sems.allocated().values()))
        nc.all_engine_barrier()


def _attention_body(nc, tc, ctx):
    x_e = nc.dram_tensor("x", [C, N], F32, kind="ExternalInput")
    wqt2_e = nc.dram_tensor("wqt2", [C, 128], F32, kind="ExternalInput")
    wkt2_e = nc.dram_tensor("wkt2", [C, 128], F32, kind="ExternalInput")
    wvt_e = nc.dram_tensor("wvt", [C, C], F32, kind="ExternalInput")
    bq2_e = nc.dram_tensor("bq2", [128, 1], F32, kind="ExternalInput")
    bk2_e = nc.dram_tensor("bk2", [128, 1], F32, kind="ExternalInput")
    bv_e = nc.dram_tensor("bv2", [128, CT], F32, kind="ExternalInput")
    gamma_e = nc.dram_tensor("gamma128", [128, 1], F32, kind="ExternalInput")
    mneg_e = nc.dram_tensor("mneg", [1, N], F32, kind="ExternalInput")
    out_e = nc.dram_tensor("out", [C, N], F32, kind="ExternalOutput")

    x_v = x_e.rearrange("(t p) n -> p t n", p=128)      # [128, CT, N]
    out_v = out_e.rearrange("(t p) n -> p t n", p=128)  # [128, CT, N]
    wqt_v = wqt2_e.rearrange("(t p) m -> p t m", p=128)
    wkt_v = wkt2_e.rearrange("(t p) m -> p t m", p=128)
    wvt_v = wvt_e.rearrange("(t p) m -> p t m", p=128)

    const = ctx.enter_context(tc.tile_pool(name="const", bufs=1))
    sb = ctx.enter_context(tc.tile_pool(name="sb", bufs=1))
    eps = ctx.enter_context(tc.tile_pool(name="eps", bufs=4))
    outp = ctx.enter_context(tc.tile_pool(name="outp", bufs=2))

    # ---- constants / weights ----
    bq2 = const.tile([128, 1], F32)
    bk2 = const.tile([128, 1], F32)
    bv2 = const.tile([128, CT], F32)
    gamma = const.tile([128, 1], F32)
    nc.gpsimd.dma_start(out=bq2, in_=bq2_e[:, :])
    nc.gpsimd.dma_start(out=bk2, in_=bk2_e[:, :])
    nc.gpsimd.dma_start(out=bv2, in_=bv_e[:, :])
    nc.gpsimd.dma_start(out=gamma, in_=gamma_e[:, :])

    wq_f = const.tile([128, CT, 128], F32)
    wk_f = const.tile([128, CT, 128], F32)
    wv_f = const.tile([128, CT, C], F32)
    nc.scalar.dma_start(out=wq_f, in_=wqt_v)
    nc.scalar.dma_start(out=wk_f, in_=wkt_v)
    nc.scalar.dma_start(out=wv_f, in_=wvt_v)
    wq_r = const.tile([128, CT, 128], F32R)
    wk_r = const.tile([128, CT, 128], F32R)
    wv_r = const.tile([128, CT, C], F32R)
    nc.vector.tensor_copy(out=wq_r, in_=wq_f)
    nc.vector.tensor_copy(out=wk_r, in_=wk_f)
    nc.vector.tensor_copy(out=wv_r, in_=wv_f)

    ident = const.tile([128, 128], BF16)
    make_identity(nc, ident)

    # ---- x load + fp32r round + projections, pipelined in 512-col chunks ----
    x_sb = sb.tile([128, CT, N], F32)
    xf_r = sb.tile([128, CT, N], F32R)
    qT = sb.tile([128, N], F32R)   # q~^T: q rows 0-31/64-95, -m row 32/96
    kT = sb.tile([128, N], F32R)   # k~^T: k rows 0-31/64-95, ones row 32/96
    v1T = sb.tile([128, JT, C + 1], F8E4)  # [j-part, j-tile, c | ones]
    E = sb.tile([128, JT, IB], F8E5)  # exp(S~^T) for one i-block
    E_u8 = E.bitcast(U8)

    # ACT exp-table preload: dummy exp (output overwritten by the x_sb
    # load, which gives the location a reader) pulls the ~2.7us table DMA
    # into the input-load window instead of stalling the first softmax
    zt = const.tile([128, 1], F32)
    nc.vector.memset(zt, 0.0)
    nc.scalar.activation(
        out=x_sb[:, 0, 0:1], in_=zt, func=mybir.ActivationFunctionType.Exp
    )

    with tc.tile_pool(name="psA", bufs=6, space="PSUM") as psA:
        # HAM warm-up: ~4us of dependency-free back-to-back matmuls so the
        # PE clock gate opens (1.2 -> 2.4 GHz) before the real work lands
        wu = const.tile([128, 512], BF16)
        nc.vector.memset(wu, 0.0)
        pwu = psA.tile([128, 512], F32, tag="pj", name="pwu")
        for _ in range(9):
            nc.tensor.matmul(
                pwu, wu[:, 0:128], wu[:, 0:512], start=True, stop=True
            )
        for ch in range(16):
            # 256-col chunks: first data reaches the PE ~5us sooner than
            # 512-col chunks (the first transfer is DMA-cold and slow),
            # closing the post-warm-up idle gap that re-throttles the HAM
            sl = bass.ts(ch, 256)
            nc.sync.dma_start(out=xf_r[:, :, sl], in_=x_v[:, :, sl].bitcast(F32R))
            pq = psA.tile([128, 256], F32, tag="pj")
            nc.tensor.matmul(pq, wq_r[:, 0, :], xf_r[:, 0, sl], start=True, stop=False)
            nc.tensor.matmul(pq, wq_r[:, 1, :], xf_r[:, 1, sl], start=False, stop=True)
            nc.vector.tensor_scalar(
                out=qT[:, sl], in0=pq, scalar1=bq2, scalar2=None,
                op0=mybir.AluOpType.add,
            )
            pk = psA.tile([128, 256], F32, tag="pj")
            nc.tensor.matmul(pk, wk_r[:, 0, :], xf_r[:, 0, sl], start=True, stop=False)
            nc.tensor.matmul(pk, wk_r[:, 1, :], xf_r[:, 1, sl], start=False, stop=True)
            nc.vector.tensor_scalar(
                out=kT[:, sl], in0=pk, scalar1=bk2, scalar2=None,
                op0=mybir.AluOpType.add,
            )
            for nt in range(ch * 2, ch * 2 + 2):
                pv = psA.tile([128, C], F32, tag="pj")
                nc.tensor.matmul(
                    pv, xf_r[:, 0, bass.ts(nt, 128)], wv_r[:, 0, :],
                    start=True, stop=False,
                )
                nc.tensor.matmul(
                    pv, xf_r[:, 1, bass.ts(nt, 128)], wv_r[:, 1, :],
                    start=False, stop=True,
                )
                nc.scalar.copy(out=v1T[:, nt, 0:C], in_=pv)
        # fp8 memset is rejected by the ISA checker; convert-copy from f32
        ones32 = const.tile([128, JT, 1], F32)
        nc.vector.memset(ones32, 1.0)
        nc.vector.tensor_copy(out=v1T[:, :, C : C + 1], in_=ones32)

    # augmented rows: -m (per-query shift, host-computed) and ones
    nc.sync.dma_start(out=qT[32:33, :], in_=mneg_e[:, :].bitcast(F32R))
    nc.sync.dma_start(out=qT[96:97, :], in_=mneg_e[:, :].bitcast(F32R))
    kT_f32 = kT.bitcast(F32)
    nc.vector.memset(kT_f32[32:33, :], 1.0)
    nc.vector.memset(kT_f32[96:97, :], 1.0)

    # residual load: off the critical path, overlaps early attention work
    nc.sync.dma_start(out=x_sb, in_=x_v)

    # xb = x + gamma*bv  (residual with bv folded in; written in place)
    gbv = const.tile([128, CT], F32)
    nc.vector.tensor_scalar(
        out=gbv, in0=bv2, scalar1=gamma, scalar2=None, op0=mybir.AluOpType.mult
    )
    for t in range(CT):
        nc.vector.tensor_scalar(
            out=x_sb[:, t, :], in0=x_sb[:, t, :], scalar1=gbv[:, t : t + 1],
            scalar2=None, op0=mybir.AluOpType.add,
        )

    # ---- attention ----
    def emit_energy(ib, jg):
        # S~^T for 4 key-tiles (K=64 augmented matmuls, 2-way row groups);
        # exp in 2 halves so PV can start on the first pair of key-tiles
        # while the second is still in the ACT/DVE pipe
        isl = bass.ds(ib * IB, IB)
        halves = [
            psS.tile([128, JGRP // 2, IB], F32, tag=f"S{h}", name=f"S_{ib}_{jg}_{h}")
            for h in range(2)
        ]
        for g in range(JGRP):
            jt = jg * JGRP + g
            gp = bass.ds(64 * (g % 2), 64)
            nc.tensor.matmul(
                halves[g // 2][:, g % 2, :],
                kT[gp, bass.ts(jt, 128)],
                qT[gp, isl],
                start=True, stop=True,
                tile_position=(64 * (g % 2), 0),
            )
        for h in range(2):
            t_idx = jg * 2 + h
            jts = jg * JGRP + h * 2
            if t_idx in DVE_TILES:
                nc.vector.tensor_scalar(
                    out=E_u8[:, jts : jts + 2, :],
                    in0=halves[h][:, :, :],
                    scalar1=SCHRAU_A, scalar2=SCHRAU_B,
                    op0=mybir.AluOpType.mult, op1=mybir.AluOpType.add,
                )
            else:
                nc.scalar.activation(
                    out=E[:, jts : jts + 2, :],
                    in_=halves[h][:, :, :],
                    func=mybir.ActivationFunctionType.Exp,
                )

    with (
        tc.tile_pool(name="psS", bufs=1, space="PSUM") as psS,
        tc.tile_pool(name="psO", bufs=4, space="PSUM") as psO,
    ):
        emit_energy(0, 0)
        for ib in range(N_IB):
            po = [
                psO.tile([128, C + 1], F32, tag="acc", name=f"po_{ib}_{i_s}")
                for i_s in range(4)
            ]
            for jg in range(N_JG):
                # software pipeline: queue the NEXT group's energy+exp ahead
                # of this group's PV matmuls so ACT/DVE overlap the PE stream
                if jg + 1 < N_JG:
                    emit_energy(ib, jg + 1)
                elif ib + 1 < N_IB:
                    emit_energy(ib + 1, 0)
                for pr in range(JGRP // 2):
                    jt = jg * JGRP + pr * 2
                    for i_s in range(4):
                        nc.tensor.matmul(
                            po[i_s],
                            E[:, jt : jt + 2, bass.ts(i_s, 128)],
                            v1T[:, jt : jt + 2, :],
                            start=(jt == 0), stop=(jt == JT - 2),
                            perf_mode=mybir.MatmulPerfMode.DoubleRow,
                        )
            # epilogue: normalize, transpose to [c, n], residual; one
            # batched store per (i-block, channel-tile) instead of eight
            # small ones (each dma_start costs ~660ns of sync-queue issue
            # time, which otherwise serializes into the kernel tail)
            ot = outp.tile([128, CT, IB], F32, tag="ot")
            for i_s in range(4):
                rd = eps.tile([128, 1], F32, tag="rd")
                nc.vector.reciprocal(out=rd, in_=po[i_s][:, C : C + 1])
                nc.vector.tensor_mul(out=rd, in0=rd, in1=gamma)
                pvn = eps.tile([128, C], BF16, tag="pvn")
                nc.vector.tensor_scalar(
                    out=pvn, in0=po[i_s][:, 0:C], scalar1=rd, scalar2=None,
                    op0=mybir.AluOpType.mult,
                )
                pt = psO.tile([128, C], BF16, tag="acc")
                nc.tensor.transpose(pt[:, 0:128], pvn[:, 0:128], ident)
                nc.tensor.transpose(pt[:, 128:256], pvn[:, 128:256], ident)
                for t in range(CT):
                    nc.vector.tensor_add(
                        out=ot[:, t, bass.ts(i_s, 128)],
                        in0=pt[:, bass.ts(t, 128)],
                        in1=x_sb[:, t, bass.ds(ib * IB + i_s * 128, 128)],
                    )
            for t in range(CT):
                nc.sync.dma_start(
                    out=out_v[:, t, bass.ts(ib, IB)], in_=ot[:, t, :]
                )


_CACHE = {}


def _build():
    if "nc" not in _CACHE:
        nc = bass.Bass()
        from contextlib import ExitStack
        with PatchedTileContext(nc) as tc, ExitStack() as ctx:
            _attention_body(nc, tc, ctx)
        _CACHE["nc"] = nc
    return _CACHE["nc"]


def _host_shift(x, wq, bq, wk, bk):
    """Per-query exp shift m (one [1, N] row per batch). Only numerics
    depend on it (it cancels in softmax): m must sit within about
    [rowmax - 10, rowmax + 8] of each query's true row max so that
    exp(S - m) fits e5m2's dynamic range. Built from cheap host features:
    sample maxes over the 256 largest-|k| columns and a 16-strided comb,
    plus a |q|-norm linear fit."""
    xf = x.reshape(B, C, N).astype(np.float32)
    q = np.einsum('oc,bcn->bno', wq.astype(np.float32), xf) + bq.astype(np.float32)
    k = np.einsum('oc,bcn->bon', wk.astype(np.float32), xf) \
        + bk.astype(np.float32)[None, :, None]
    kn = np.linalg.norm(k, axis=1)
    idx = np.argpartition(-kn, 256, axis=-1)[:, :256]
    ksel = np.take_along_axis(k, idx[:, None, :], axis=2)
    topk = np.einsum('bno,bom->bnm', q, ksel).max(-1)
    smax16 = np.einsum('bno,bom->bnm', q, k[:, :, ::16]).max(-1)
    qn = np.linalg.norm(q, axis=-1)
    fit = 3.916 * qn - 0.737
    return np.maximum.reduce([topk + 0.5, smax16 + 0.5, fit + 2.0])


def _prep_in_maps(x, wq, bq, wk, bk, wv, bv, gamma):
    asc = np.ascontiguousarray
    z32 = np.zeros((32, C), np.float32)
    wqt2 = asc(np.concatenate([wq, z32, wq, z32]).T.astype(np.float32))  # [C,128]
    wkt2 = asc(np.concatenate([wk, z32, wk, z32]).T.astype(np.float32))
    wvt = asc(wv.T.astype(np.float32))                      # [C, C]
    bz = np.zeros(32, np.float32)
    bq2 = asc(np.concatenate([bq, bz, bq, bz])[:, None].astype(np.float32))
    bk2 = asc(np.concatenate([bk, bz, bk, bz])[:, None].astype(np.float32))
    bv2 = asc(bv.reshape(CT, 128).T.astype(np.float32))     # [128, CT]
    g128 = np.full((128, 1), np.float32(gamma[0]), dtype=np.float32)
    m = _host_shift(x, wq, bq, wk, bk)                      # [B, N]
    maps = []
    for b in range(B):
        maps.append({
            "x": asc(x[b].reshape(C, N).astype(np.float32)),
            "wqt2": wqt2, "wkt2": wkt2, "wvt": wvt,
            "bq2": bq2, "bk2": bk2, "bv2": bv2, "gamma128": g128,
            "mneg": asc(-m[b][None, :].astype(np.float32)),
        })
    return maps


def _run(inputs, trace=False):
    nc = _build()
    in_maps = _prep_in_maps(**{k: np.asarray(v) for k, v in inputs.items()})
    res = run_bass_kernel_spmd(nc, in_maps, list(range(NCORES)), trace=trace)
    out = np.stack([res.results[b]["out"].reshape(C, H, W) for b in range(B)])
    return out.astype(np.float32), res


def kernel(**inputs):
    out, _ = _run(inputs, trace=False)
    return out


# revision 12
# speedup vs baseline: 1.1652x; 1.1652x over previous
"""Multi-head self-attention (1x1-conv projections, N=4096 spatial tokens,
C=256 channels, Cq=32) on 8 TRN2 NeuronCores, data-parallel over batch.

Per core (one batch element, x as [C, N]):
  q = wq @ x + bq          [Cq, N]
  k = wk @ x + bk          [Cq, N]
  v = wv @ x               [C, N]   (bv folded into the epilogue)
  S = q^T k                [N, N]
  P = softmax(S, axis=-1)
  out = gamma * (v @ P^T + bv) + x

Layout strategy: compute S^T tiles (keys j on partitions, queries i on the
free dim) so softmax's exp output E^T feeds the PV matmul as the stationary
operand with rhs = [v^T | ones]; the ones column accumulates the softmax
denominator for free (no P transposes, no separate reduction).

fp8 fast path: E is stored as fp8e5 (e5m2) and v as fp8e4 (e4m3) so the PV
matmul runs in DoubleRow perf mode (two key-tiles contracted per
instruction, 2x PE throughput vs bf16). e5m2's ~22-e-folding dynamic range
requires a per-query shift m_i: exp(S_ij - m_i). The shift cancels exactly
in softmax, so m_i only needs to track the row max within ~[-8, +10]; it
is computed ON THE HOST (cheap q/k projections + top-|k|-column and strided
sample maxes + a |q| linear fit) and folded into the energy matmul by
augmenting the contraction dim: q~ = [q; -m; 0...], k~ = [k; 1; 0...]
(K=32 -> 64, which is free on the PE since matmul cost is output-bound).

exp is split between the ACT engine (true exp, fp8e5 output) and the DVE
(Schraudolph bit-trick: bits = round(S~ * 4/ln2 + 59.7) saturating-cast to
uint8, bitcast as e5m2), sized so neither engine bottlenecks. The DVE cast
saturates negatives to 0, which implements exp underflow for free.

dtypes: fp32r (tf32-like, full PE speed at moving-dim>=256) for the
q/k/energy path where exp amplifies absolute error; fp8 for the P*V path
where softmax normalization cancels it.
"""

import numpy as np
import ml_dtypes

import concourse.bass as bass
import concourse.mybir as mybir
import concourse.tile as tile
from concourse.bass_utils import run_bass_kernel_spmd
from concourse.masks import make_identity
from concourse.tile import ScopedClock

F32 = mybir.dt.float32
F32R = mybir.dt.float32r
BF16 = mybir.dt.bfloat16
F8E5 = mybir.dt.float8e5
F8E4 = mybir.dt.float8e4
U8 = mybir.dt.uint8

B, C, CQ = 8, 256, 32
H = W = 64
N = H * W            # 4096 tokens
NCORES = 8
CT = C // 128        # 2 channel tiles
IB = 512             # queries per i-block
N_IB = N // IB       # 8
JT = N // 128        # 32 key tiles
JGRP = 4             # key tiles per exp group (one PSUM S tile pair = 4 banks)
N_JG = JT // JGRP    # 8

# Schraudolph exp-to-e5m2 constants for the DVE share of the softmax
SCHRAU_A = 5.7708017  # 4 / ln 2
SCHRAU_B = 59.7       # 4 * 15 (e5m2 bias) - rounding correction
# which of the 16 S~ PSUM half-tiles per i-block go to the DVE (rest: ACT)
import os as _os
DVE_TILES = (frozenset() if _os.environ.get("KNODVE")
             else frozenset(range(1, 16, 2)))


class PatchedTileContext(tile.TileContext):
    """This walrus build supports only ONE sync-wait command per
    instruction. Peel extra waits into standalone single-wait NOPs on the
    same engine queue, emitted immediately before the instruction (a serial
    conjunction of waits - semantically identical). Same treatment for the
    kernel-tail drain, whose global-clock waits otherwise all land on one
    Drain instruction."""

    MAX_WAITS_PER_INST = 1

    def _add_instruction(self, inst):
        si = inst.sync_info
        waits = list(si.on_wait) if si is not None and si.on_wait else []
        if len(waits) > self.MAX_WAITS_PER_INST and inst.engine is not None:
            keep = waits[-self.MAX_WAITS_PER_INST:]
            peel = waits[: -self.MAX_WAITS_PER_INST]
            for w in peel:
                nop = mybir.InstNoOp(
                    name=self.nc.get_next_instruction_name(),
                    ins=[],
                    outs=[],
                    sync_info=mybir.SyncInfo(on_wait=[w], on_update=[]),
                )
                nop.engine = inst.engine
                super()._add_instruction(nop)
            inst.sync_info = mybir.SyncInfo(
                on_wait=keep,
                on_update=list(si.on_update) if si.on_update else [],
            )
        super()._add_instruction(inst)

    def _drain_and_barrier(self, tick_clock, wait_clock):
        nc = self.nc
        carrier = nc.sync.nop()
        wait_clock.add_sem_waits(
            carrier.ins, ScopedClock({None: tick_clock.global_clock})
        )
        si = carrier.ins.sync_info
        waits = list(si.on_wait) if si is not None and si.on_wait else []
        carrier.ins.sync_info = None
        for w in waits:
            h = bass.SemaphoreHandle(name=w.ant_name or f"sem{w.id}", num=w.id)
            if w.wait_mode == "sem-ge-imm":
                nc.sync.wait_ge(h, w.wait_value)
            else:
                op = {
                    "sem-eq-imm": "eq",
                    "sem-le-imm": "le",
                    "sem-lt-imm": "lt",
                    "sem-gt-imm": "gt",
                }[w.wait_mode]
                nc.sync.wait_op(h, w.wait_value, op)
        nc.sync.drain()
        nc.all_engine_barrier()
        assert self.sems is not None
        popped = nc._tile_sem_poison_stack.pop()
        assert popped is self._sem_poison
        nc.clear_and_free_semaphores(list(self.sems.allocated().values()))
        nc.all_engine_barrier()


def _attention_body(nc, tc, ctx):
    x_e = nc.dram_tensor("x", [C, N], F32, kind="ExternalInput")
    wqt2_e = nc.dram_tensor("wqt2", [C, 64], F32, kind="ExternalInput")
    wkt2_e = nc.dram_tensor("wkt2", [C, 64], F32, kind="ExternalInput")
    wvt_e = nc.dram_tensor("wvt", [C, C], F32, kind="ExternalInput")
    bq2_e = nc.dram_tensor("bq2", [64, 1], F32, kind="ExternalInput")
    bk2_e = nc.dram_tensor("bk2", [64, 1], F32, kind="ExternalInput")
    bv_e = nc.dram_tensor("bv2", [128, CT], F32, kind="ExternalInput")
    gamma_e = nc.dram_tensor("gamma128", [128, 1], F32, kind="ExternalInput")
    mneg_e = nc.dram_tensor("mneg", [1, N], mybir.dt.uint16, kind="ExternalInput")
    out_e = nc.dram_tensor("out", [C, N], F32, kind="ExternalOutput")
    import os
    DBG = bool(os.environ.get("KDBG"))
    if DBG:
        qTd_e = nc.dram_tensor("qTd", [64, N], mybir.dt.uint16, kind="ExternalOutput")
        kTd_e = nc.dram_tensor("kTd", [64, N], mybir.dt.uint16, kind="ExternalOutput")
        v1d_e = nc.dram_tensor("v1d", [128, JT * (C + 1)], U8, kind="ExternalOutput")
        E0d_e = nc.dram_tensor("E0d", [128, JT * IB], U8, kind="ExternalOutput")

    x_v = x_e.rearrange("(t p) n -> p t n", p=128)      # [128, CT, N]
    out_v = out_e.rearrange("(t p) n -> p t n", p=128)  # [128, CT, N]
    wqt_v = wqt2_e.rearrange("(t p) m -> p t m", p=128)
    wkt_v = wkt2_e.rearrange("(t p) m -> p t m", p=128)
    wvt_v = wvt_e.rearrange("(t p) m -> p t m", p=128)

    const = ctx.enter_context(tc.tile_pool(name="const", bufs=1))
    sb = ctx.enter_context(tc.tile_pool(name="sb", bufs=1))
    eps = ctx.enter_context(tc.tile_pool(name="eps", bufs=4))
    outp = ctx.enter_context(tc.tile_pool(name="outp", bufs=2))

    # ---- constants / weights ----
    bq2 = const.tile([64, 1], F32)
    bk2 = const.tile([64, 1], F32)
    bv2 = const.tile([128, CT], F32)
    gamma = const.tile([128, 1], F32)
    nc.gpsimd.dma_start(out=bq2, in_=bq2_e[:, :])
    nc.gpsimd.dma_start(out=bk2, in_=bk2_e[:, :])
    nc.gpsimd.dma_start(out=bv2, in_=bv_e[:, :])
    nc.gpsimd.dma_start(out=gamma, in_=gamma_e[:, :])

    wq_f = const.tile([128, CT, 64], F32)
    wk_f = const.tile([128, CT, 64], F32)
    wv_f = const.tile([128, CT, C], F32)
    nc.scalar.dma_start(out=wq_f, in_=wqt_v)
    nc.scalar.dma_start(out=wk_f, in_=wkt_v)
    nc.scalar.dma_start(out=wv_f, in_=wvt_v)
    wq_r = const.tile([128, CT, 64], F32R)
    wk_r = const.tile([128, CT, 64], F32R)
    wv_r = const.tile([128, CT, C], F32R)
    nc.vector.tensor_copy(out=wq_r, in_=wq_f)
    nc.vector.tensor_copy(out=wk_r, in_=wk_f)
    nc.vector.tensor_copy(out=wv_r, in_=wv_f)

    ident = const.tile([128, 128], BF16)
    make_identity(nc, ident)

    # ---- x load + fp32r round + projections, pipelined in 512-col chunks ----
    x_sb = sb.tile([128, CT, N], F32)
    xf_r = sb.tile([128, CT, N], F32R)
    qT = sb.tile([64, N], BF16)   # q~^T: q rows 0-31, -m row 32
    kT = sb.tile([64, N], BF16)   # k~^T: k rows 0-31, ones row 32
    v1T = sb.tile([128, JT, C + 1], F8E4)  # [j-part, j-tile, c | ones]
    E0 = sb.tile([128, JT, IB], F8E5)  # exp(S~^T), double-buffered by ib parity
    E1 = sb.tile([128, JT, IB], F8E5)
    E_PAR = (E0, E1)
    E_U8 = (E0.bitcast(U8), E1.bitcast(U8))

    # ACT exp-table preload: dummy exp (output overwritten by the x_sb
    # load, which gives the location a reader) pulls the ~2.7us table DMA
    # into the input-load window instead of stalling the first softmax
    zt = const.tile([128, 1], F32)
    nc.vector.memset(zt, 0.0)
    nc.scalar.activation(
        out=x_sb[:, 0, 0:1], in_=zt, func=mybir.ActivationFunctionType.Exp
    )

    with tc.tile_pool(name="psA", bufs=6, space="PSUM") as psA:
        # HAM warm-up: ~4us of dependency-free back-to-back matmuls so the
        # PE clock gate opens (1.2 -> 2.4 GHz) before the real work lands
        wu = const.tile([128, 512], BF16)
        nc.vector.memset(wu, 0.0)
        pwu = psA.tile([128, 512], F32, tag="pj", name="pwu")
        for _ in range(9):
            nc.tensor.matmul(
                pwu, wu[:, 0:128], wu[:, 0:512], start=True, stop=True
            )
        for ch in range(16):
            # 256-col chunks: first data reaches the PE ~5us sooner than
            # 512-col chunks (the first transfer is DMA-cold and slow),
            # closing the post-warm-up idle gap that re-throttles the HAM
            sl = bass.ts(ch, 256)
            nc.sync.dma_start(out=xf_r[:, :, sl], in_=x_v[:, :, sl].bitcast(F32R))
            pq = psA.tile([64, 256], F32, tag="pj")
            nc.tensor.matmul(pq, wq_r[:, 0, :], xf_r[:, 0, sl], start=True, stop=False)
            nc.tensor.matmul(pq, wq_r[:, 1, :], xf_r[:, 1, sl], start=False, stop=True)
            nc.vector.tensor_scalar(
                out=qT[:, sl], in0=pq, scalar1=bq2, scalar2=None,
                op0=mybir.AluOpType.add,
            )
            pk = psA.tile([64, 256], F32, tag="pj")
            nc.tensor.matmul(pk, wk_r[:, 0, :], xf_r[:, 0, sl], start=True, stop=False)
            nc.tensor.matmul(pk, wk_r[:, 1, :], xf_r[:, 1, sl], start=False, stop=True)
            nc.vector.tensor_scalar(
                out=kT[:, sl], in0=pk, scalar1=bk2, scalar2=None,
                op0=mybir.AluOpType.add,
            )
            for nt in range(ch * 2, ch * 2 + 2):
                pv = psA.tile([128, C], F32, tag="pj")
                nc.tensor.matmul(
                    pv, xf_r[:, 0, bass.ts(nt, 128)], wv_r[:, 0, :],
                    start=True, stop=False,
                )
                nc.tensor.matmul(
                    pv, xf_r[:, 1, bass.ts(nt, 128)], wv_r[:, 1, :],
                    start=False, stop=True,
                )
                nc.scalar.copy(out=v1T[:, nt, 0:C], in_=pv)
        # fp8 memset is rejected by the ISA checker; convert-copy from f32
        ones32 = const.tile([128, JT, 1], F32)
        nc.vector.memset(ones32, 1.0)
        nc.vector.tensor_copy(out=v1T[:, :, C : C + 1], in_=ones32)

    # augmented rows: -m (per-query shift, host-computed) and ones
    nc.sync.dma_start(out=qT[32:33, :], in_=mneg_e[:, :].bitcast(BF16))
    nc.vector.memset(kT[32:33, :], 1.0)

    # residual load: off the critical path, overlaps early attention work
    nc.sync.dma_start(out=x_sb, in_=x_v)

    # xb = x + gamma*bv  (residual with bv folded in; written in place)
    gbv = const.tile([128, CT], F32)
    nc.vector.tensor_scalar(
        out=gbv, in0=bv2, scalar1=gamma, scalar2=None, op0=mybir.AluOpType.mult
    )
    for t in range(CT):
        nc.vector.tensor_scalar(
            out=x_sb[:, t, :], in0=x_sb[:, t, :], scalar1=gbv[:, t : t + 1],
            scalar2=None, op0=mybir.AluOpType.add,
        )

    # ---- attention ----
    def emit_energy(ib, jg):
        # S~^T for 4 key-tiles (K=33 augmented matmuls, uniform PE config);
        # exp in 2 halves so PV can start on the first pair of key-tiles
        # while the second is still in the ACT/DVE pipe
        isl = bass.ds(ib * IB, IB)
        E, E_u8 = E_PAR[ib % 2], E_U8[ib % 2]
        halves = [
            psS.tile([128, JGRP // 2, IB], F32, tag="S", name=f"S_{ib}_{jg}_{h}")
            for h in range(2)
        ]
        for g in range(JGRP):
            jt = jg * JGRP + g
            nc.tensor.matmul(
                halves[g // 2][:, g % 2, :],
                kT[:, bass.ts(jt, 128)],
                qT[:, isl],
                start=True, stop=True,
            )
        for h in range(2):
            t_idx = jg * 2 + h
            jts = jg * JGRP + h * 2
            if t_idx in DVE_TILES:
                nc.vector.tensor_scalar(
                    out=E_u8[:, jts : jts + 2, :],
                    in0=halves[h][:, :, :],
                    scalar1=SCHRAU_A, scalar2=SCHRAU_B,
                    op0=mybir.AluOpType.mult, op1=mybir.AluOpType.add,
                )
            else:
                nc.scalar.activation(
                    out=E[:, jts : jts + 2, :],
                    in_=halves[h][:, :, :],
                    func=mybir.ActivationFunctionType.Exp,
                )

    def emit_pv(ib, jg, po2, base_i_s):
        E = E_PAR[ib % 2]
        for pr in range(JGRP // 2):
            jt = jg * JGRP + pr * 2
            for i_s in (base_i_s, base_i_s + 1):
                nc.tensor.matmul(
                    po2[i_s - base_i_s],
                    E[:, jt : jt + 2, bass.ts(i_s, 128)],
                    v1T[:, jt : jt + 2, :],
                    start=(jt == 0), stop=(jt == JT - 2),
                    perf_mode=mybir.MatmulPerfMode.DoubleRow,
                )

    def emit_epilogue(ib, po2, base_i_s, ot):
        for i_s in (base_i_s, base_i_s + 1):
            po_t = po2[i_s - base_i_s]
            rd = eps.tile([128, 1], F32, tag="rd")
            nc.vector.reciprocal(out=rd, in_=po_t[:, C : C + 1])
            nc.vector.tensor_mul(out=rd, in0=rd, in1=gamma)
            pvn = eps.tile([128, C], BF16, tag="pvn")
            nc.vector.tensor_scalar(
                out=pvn, in0=po_t[:, 0:C], scalar1=rd, scalar2=None,
                op0=mybir.AluOpType.mult,
            )
            pt = psS.tile([128, C], BF16, tag="S")
            nc.tensor.transpose(pt[:, 0:128], pvn[:, 0:128], ident)
            nc.tensor.transpose(pt[:, 128:256], pvn[:, 128:256], ident)
            for t in range(CT):
                nc.vector.tensor_add(
                    out=ot[:, t, bass.ts(i_s, 128)],
                    in0=pt[:, bass.ts(t, 128)],
                    in1=x_sb[:, t, bass.ds(ib * IB + i_s * 128, 128)],
                )

    with (
        tc.tile_pool(name="psS", bufs=3, space="PSUM") as psS,
        tc.tile_pool(name="psO", bufs=2, space="PSUM") as psO,
    ):
        emit_energy(0, 0)
        for ib in range(N_IB):
            # first i-half: PV for i_s 0,1 pipelined with energy+exp of the
            # next group; second i-half afterwards as always-ready PE work
            # that absorbs exp-latency jitter (keeps the HAM clock up)
            po01 = [
                psO.tile([128, C + 1], F32, tag="acc", name=f"po_{ib}_{i_s}")
                for i_s in range(2)
            ]
            for jg in range(N_JG):
                if jg + 1 < N_JG:
                    emit_energy(ib, jg + 1)
                elif ib + 1 < N_IB:
                    emit_energy(ib + 1, 0)
                emit_pv(ib, jg, po01, 0)
            ot = outp.tile([128, CT, IB], F32, tag="ot")
            emit_epilogue(ib, po01, 0, ot)
            po23 = [
                psO.tile([128, C + 1], F32, tag="acc", name=f"po_{ib}_{i_s + 2}")
                for i_s in range(2)
            ]
            for jg in range(N_JG):
                emit_pv(ib, jg, po23, 2)
            emit_epilogue(ib, po23, 2, ot)
            # one batched store per (i-block, channel-tile): each dma_start
            # costs ~660ns of sync-queue issue time
            for t in range(CT):
                nc.sync.dma_start(
                    out=out_v[:, t, bass.ts(ib, IB)], in_=ot[:, t, :]
                )
        if DBG:
            nc.sync.dma_start(out=qTd_e[:, :], in_=qT.bitcast(mybir.dt.uint16))
            nc.sync.dma_start(out=kTd_e[:, :], in_=kT.bitcast(mybir.dt.uint16))
            nc.sync.dma_start(out=v1d_e[:, :], in_=v1T.bitcast(U8).rearrange("p a b -> p (a b)"))
            nc.sync.dma_start(out=E0d_e[:, :], in_=E0.bitcast(U8).rearrange("p a b -> p (a b)"))


_CACHE = {}


def _build():
    if "nc" not in _CACHE:
        nc = bass.Bass()
        from contextlib import ExitStack
        with PatchedTileContext(nc) as tc, ExitStack() as ctx:
            _attention_body(nc, tc, ctx)
        _CACHE["nc"] = nc
    return _CACHE["nc"]


def _host_shift(x, wq, bq, wk, bk):
    """Per-query exp shift m (one [1, N] row per batch). Only numerics
    depend on it (it cancels in softmax): m must sit within about
    [rowmax - 10, rowmax + 8] of each query's true row max so that
    exp(S - m) fits e5m2's dynamic range. Built from cheap host features:
    sample maxes over the 256 largest-|k| columns and a 16-strided comb,
    plus a |q|-norm linear fit."""
    xf = x.reshape(B, C, N).astype(np.float32)
    q = np.einsum('oc,bcn->bno', wq.astype(np.float32), xf) + bq.astype(np.float32)
    k = np.einsum('oc,bcn->bon', wk.astype(np.float32), xf) \
        + bk.astype(np.float32)[None, :, None]
    kn = np.linalg.norm(k, axis=1)
    idx = np.argpartition(-kn, 256, axis=-1)[:, :256]
    ksel = np.take_along_axis(k, idx[:, None, :], axis=2)
    topk = np.einsum('bno,bom->bnm', q, ksel).max(-1)
    smax16 = np.einsum('bno,bom->bnm', q, k[:, :, ::16]).max(-1)
    qn = np.linalg.norm(q, axis=-1)
    fit = 3.916 * qn - 0.737
    return np.maximum.reduce([topk + 0.5, smax16 + 0.5, fit + 2.0])


def _prep_in_maps(x, wq, bq, wk, bk, wv, bv, gamma):
    asc = np.ascontiguousarray
    z32 = np.zeros((32, C), np.float32)
    wqt2 = asc(np.concatenate([wq, z32]).T.astype(np.float32))  # [C, 64]
    wkt2 = asc(np.concatenate([wk, z32]).T.astype(np.float32))
    wvt = asc(wv.T.astype(np.float32))                      # [C, C]
    bz = np.zeros(32, np.float32)
    bq2 = asc(np.concatenate([bq, bz])[:, None].astype(np.float32))
    bk2 = asc(np.concatenate([bk, bz])[:, None].astype(np.float32))
    bv2 = asc(bv.reshape(CT, 128).T.astype(np.float32))     # [128, CT]
    g128 = np.full((128, 1), np.float32(gamma[0]), dtype=np.float32)
    m = _host_shift(x, wq, bq, wk, bk)                      # [B, N]
    maps = []
    for b in range(B):
        maps.append({
            "x": asc(x[b].reshape(C, N).astype(np.float32)),
            "wqt2": wqt2, "wkt2": wkt2, "wvt": wvt,
            "bq2": bq2, "bk2": bk2, "bv2": bv2, "gamma128": g128,
            "mneg": asc(np.frombuffer(
                (-m[b]).astype(np.float32).astype(ml_dtypes.bfloat16).tobytes(),
                dtype=np.uint16).reshape(1, N).copy()),
        })
    return maps


def _run(inputs, trace=False):
    nc = _build()
    in_maps = _prep_in_maps(**{k: np.asarray(v) for k, v in inputs.items()})
    res = run_bass_kernel_spmd(nc, in_maps, list(range(NCORES)), trace=trace)
    out = np.stack([res.results[b]["out"].reshape(C, H, W) for b in range(B)])
    return out.astype(np.float32), res


def kernel(**inputs):
    out, _ = _run(inputs, trace=False)
    return out


# revision 13
# speedup vs baseline: 1.2209x; 1.0478x over previous
"""Multi-head self-attention (1x1-conv projections, N=4096 spatial tokens,
C=256 channels, Cq=32) on 8 TRN2 NeuronCores, data-parallel over batch.

Per core (one batch element, x as [C, N]):
  q = wq @ x + bq          [Cq, N]
  k = wk @ x + bk          [Cq, N]
  v = wv @ x               [C, N]   (bv folded into the epilogue)
  S = q^T k                [N, N]
  P = softmax(S, axis=-1)
  out = gamma * (v @ P^T + bv) + x

Layout strategy: compute S^T tiles (keys j on partitions, queries i on the
free dim) so softmax's exp output E^T feeds the PV matmul as the stationary
operand with rhs = [v^T | ones]; the ones column accumulates the softmax
denominator for free (no P transposes, no separate reduction).

fp8 fast path: E is stored as fp8e5 (e5m2) and v as fp8e4 (e4m3) so the PV
matmul runs in DoubleRow perf mode (two key-tiles contracted per
instruction, 2x PE throughput vs bf16). e5m2's ~22-e-folding dynamic range
requires a per-query shift m_i: exp(S_ij - m_i). The shift cancels exactly
in softmax, so m_i only needs to track the row max within ~[-8, +10]; it
is computed ON THE HOST (cheap q/k projections + top-|k|-column and strided
sample maxes + a |q| linear fit) and folded into the energy matmul by
augmenting the contraction dim: q~ = [q; -m; 0...], k~ = [k; 1; 0...]
(K=32 -> 64, which is free on the PE since matmul cost is output-bound).

exp is split between the ACT engine (true exp, fp8e5 output) and the DVE
(Schraudolph bit-trick: bits = round(S~ * 4/ln2 + 59.7) saturating-cast to
uint8, bitcast as e5m2), sized so neither engine bottlenecks. The DVE cast
saturates negatives to 0, which implements exp underflow for free.

dtypes: fp32r (tf32-like, full PE speed at moving-dim>=256) for the
q/k/energy path where exp amplifies absolute error; fp8 for the P*V path
where softmax normalization cancels it.
"""

import numpy as np
import ml_dtypes

import concourse.bass as bass
import concourse.mybir as mybir
import concourse.tile as tile
from concourse.bass_utils import run_bass_kernel_spmd
from concourse.masks import make_identity
from concourse.tile import ScopedClock

F32 = mybir.dt.float32
F32R = mybir.dt.float32r
BF16 = mybir.dt.bfloat16
F8E5 = mybir.dt.float8e5
F8E4 = mybir.dt.float8e4
U8 = mybir.dt.uint8

B, C, CQ = 8, 256, 32
H = W = 64
N = H * W            # 4096 tokens
NCORES = 8
CT = C // 128        # 2 channel tiles
IB = 512             # queries per i-block
N_IB = N // IB       # 8
JT = N // 128        # 32 key tiles
JGRP = 4             # key tiles per exp group (one PSUM S tile pair = 4 banks)
N_JG = JT // JGRP    # 8

# Schraudolph exp-to-e5m2 constants for the DVE share of the softmax
SCHRAU_A = 5.7708017  # 4 / ln 2
SCHRAU_B = 59.7       # 4 * 15 (e5m2 bias) - rounding correction
# which of the 16 S~ PSUM half-tiles per i-block go to the DVE (rest: ACT)
import os as _os
DVE_TILES = (frozenset() if _os.environ.get("KNODVE")
             else frozenset(range(1, 16, 2)))


class PatchedTileContext(tile.TileContext):
    """This walrus build supports only ONE sync-wait command per
    instruction. Peel extra waits into standalone single-wait NOPs on the
    same engine queue, emitted immediately before the instruction (a serial
    conjunction of waits - semantically identical). Same treatment for the
    kernel-tail drain, whose global-clock waits otherwise all land on one
    Drain instruction."""

    MAX_WAITS_PER_INST = 1

    def _add_instruction(self, inst):
        si = inst.sync_info
        waits = list(si.on_wait) if si is not None and si.on_wait else []
        if len(waits) > self.MAX_WAITS_PER_INST and inst.engine is not None:
            keep = waits[-self.MAX_WAITS_PER_INST:]
            peel = waits[: -self.MAX_WAITS_PER_INST]
            for w in peel:
                nop = mybir.InstNoOp(
                    name=self.nc.get_next_instruction_name(),
                    ins=[],
                    outs=[],
                    sync_info=mybir.SyncInfo(on_wait=[w], on_update=[]),
                )
                nop.engine = inst.engine
                super()._add_instruction(nop)
            inst.sync_info = mybir.SyncInfo(
                on_wait=keep,
                on_update=list(si.on_update) if si.on_update else [],
            )
        super()._add_instruction(inst)

    def _drain_and_barrier(self, tick_clock, wait_clock):
        nc = self.nc
        carrier = nc.sync.nop()
        wait_clock.add_sem_waits(
            carrier.ins, ScopedClock({None: tick_clock.global_clock})
        )
        si = carrier.ins.sync_info
        waits = list(si.on_wait) if si is not None and si.on_wait else []
        carrier.ins.sync_info = None
        for w in waits:
            h = bass.SemaphoreHandle(name=w.ant_name or f"sem{w.id}", num=w.id)
            if w.wait_mode == "sem-ge-imm":
                nc.sync.wait_ge(h, w.wait_value)
            else:
                op = {
                    "sem-eq-imm": "eq",
                    "sem-le-imm": "le",
                    "sem-lt-imm": "lt",
                    "sem-gt-imm": "gt",
                }[w.wait_mode]
                nc.sync.wait_op(h, w.wait_value, op)
        nc.sync.drain()
        nc.all_engine_barrier()
        assert self.sems is not None
        popped = nc._tile_sem_poison_stack.pop()
        assert popped is self._sem_poison
        nc.clear_and_free_semaphores(list(self.sems.allocated().values()))
        nc.all_engine_barrier()


def _attention_body(nc, tc, ctx):
    x_e = nc.dram_tensor("x", [C, N], F32, kind="ExternalInput")
    wqt2_e = nc.dram_tensor("wqt2", [C, 32], F32, kind="ExternalInput")
    wkt2_e = nc.dram_tensor("wkt2", [C, 32], F32, kind="ExternalInput")
    wvt_e = nc.dram_tensor("wvt", [C, C], F32, kind="ExternalInput")
    bq2_e = nc.dram_tensor("bq2", [32, 1], F32, kind="ExternalInput")
    bk2_e = nc.dram_tensor("bk2", [32, 1], F32, kind="ExternalInput")
    bv_e = nc.dram_tensor("bv2", [128, CT], F32, kind="ExternalInput")
    gamma_e = nc.dram_tensor("gamma128", [128, 1], F32, kind="ExternalInput")
    qaug_e = nc.dram_tensor("qaug", [32, N], mybir.dt.uint16, kind="ExternalInput")
    kaug_e = nc.dram_tensor("kaug", [32, N], mybir.dt.uint16, kind="ExternalInput")
    out_e = nc.dram_tensor("out", [C, N], F32, kind="ExternalOutput")
    import os
    DBG = bool(os.environ.get("KDBG"))
    if DBG:
        qTd_e = nc.dram_tensor("qTd", [64, N], mybir.dt.uint16, kind="ExternalOutput")
        kTd_e = nc.dram_tensor("kTd", [64, N], mybir.dt.uint16, kind="ExternalOutput")
        v1d_e = nc.dram_tensor("v1d", [128, JT * (C + 1)], U8, kind="ExternalOutput")
        E0d_e = nc.dram_tensor("E0d", [128, JT * IB], U8, kind="ExternalOutput")

    x_v = x_e.rearrange("(t p) n -> p t n", p=128)      # [128, CT, N]
    out_v = out_e.rearrange("(t p) n -> p t n", p=128)  # [128, CT, N]
    wqt_v = wqt2_e.rearrange("(t p) m -> p t m", p=128)
    wkt_v = wkt2_e.rearrange("(t p) m -> p t m", p=128)
    wvt_v = wvt_e.rearrange("(t p) m -> p t m", p=128)

    const = ctx.enter_context(tc.tile_pool(name="const", bufs=1))
    sb = ctx.enter_context(tc.tile_pool(name="sb", bufs=1))
    eps = ctx.enter_context(tc.tile_pool(name="eps", bufs=4))
    outp = ctx.enter_context(tc.tile_pool(name="outp", bufs=2))

    # ---- constants / weights ----
    bq2 = const.tile([32, 1], F32)
    bk2 = const.tile([32, 1], F32)
    bv2 = const.tile([128, CT], F32)
    gamma = const.tile([128, 1], F32)
    nc.gpsimd.dma_start(out=bq2, in_=bq2_e[:, :])
    nc.gpsimd.dma_start(out=bk2, in_=bk2_e[:, :])
    nc.gpsimd.dma_start(out=bv2, in_=bv_e[:, :])
    nc.gpsimd.dma_start(out=gamma, in_=gamma_e[:, :])

    wq_f = const.tile([128, CT, 32], F32)
    wk_f = const.tile([128, CT, 32], F32)
    wv_f = const.tile([128, CT, C], F32)
    nc.scalar.dma_start(out=wq_f, in_=wqt_v)
    nc.scalar.dma_start(out=wk_f, in_=wkt_v)
    nc.scalar.dma_start(out=wv_f, in_=wvt_v)
    wq_r = const.tile([128, CT, 32], F32R)
    wk_r = const.tile([128, CT, 32], F32R)
    wv_r = const.tile([128, CT, C], F32R)
    nc.vector.tensor_copy(out=wq_r, in_=wq_f)
    nc.vector.tensor_copy(out=wk_r, in_=wk_f)
    nc.vector.tensor_copy(out=wv_r, in_=wv_f)

    ident = const.tile([128, 128], BF16)
    make_identity(nc, ident)

    # ---- x load + fp32r round + projections, pipelined in 512-col chunks ----
    x_sb = sb.tile([128, CT, N], F32)
    xf_r = sb.tile([128, CT, N], F32R)
    qT = sb.tile([64, N], BF16)   # q~^T: q rows 0-31, -m row 32
    kT = sb.tile([64, N], BF16)   # k~^T: k rows 0-31, ones row 32
    v1T = sb.tile([128, JT, C + 1], F8E4)  # [j-part, j-tile, c | ones]
    E0 = sb.tile([128, JT, IB], F8E5)  # exp(S~^T), double-buffered by ib parity
    E1 = sb.tile([128, JT, IB], F8E5)
    E_PAR = (E0, E1)
    E_U8 = (E0.bitcast(U8), E1.bitcast(U8))

    nc.gpsimd.dma_start(out=qT[32:64, :], in_=qaug_e[:, :].bitcast(BF16))
    nc.gpsimd.dma_start(out=kT[32:64, :], in_=kaug_e[:, :].bitcast(BF16))

    # ACT exp-table preload: dummy exp (output overwritten by the x_sb
    # load, which gives the location a reader) pulls the ~2.7us table DMA
    # into the input-load window instead of stalling the first softmax
    zt = const.tile([128, 1], F32)
    nc.vector.memset(zt, 0.0)
    nc.scalar.activation(
        out=x_sb[:, 0, 0:1], in_=zt, func=mybir.ActivationFunctionType.Exp
    )

    with tc.tile_pool(name="psA", bufs=6, space="PSUM") as psA:
        # HAM warm-up: ~4us of dependency-free back-to-back matmuls so the
        # PE clock gate opens (1.2 -> 2.4 GHz) before the real work lands
        wu = const.tile([128, 512], BF16)
        nc.vector.memset(wu, 0.0)
        pwu = psA.tile([128, 512], F32, tag="pj", name="pwu")
        for _ in range(9):
            nc.tensor.matmul(
                pwu, wu[:, 0:128], wu[:, 0:512], start=True, stop=True
            )
        for ch in range(16):
            # 256-col chunks: first data reaches the PE ~5us sooner than
            # 512-col chunks (the first transfer is DMA-cold and slow),
            # closing the post-warm-up idle gap that re-throttles the HAM
            sl = bass.ts(ch, 256)
            nc.sync.dma_start(out=xf_r[:, :, sl], in_=x_v[:, :, sl].bitcast(F32R))
            pq = psA.tile([32, 256], F32, tag="pj")
            nc.tensor.matmul(pq, wq_r[:, 0, :], xf_r[:, 0, sl], start=True, stop=False)
            nc.tensor.matmul(pq, wq_r[:, 1, :], xf_r[:, 1, sl], start=False, stop=True)
            nc.vector.tensor_scalar(
                out=qT[0:32, sl], in0=pq, scalar1=bq2, scalar2=None,
                op0=mybir.AluOpType.add,
            )
            pk = psA.tile([32, 256], F32, tag="pj")
            nc.tensor.matmul(pk, wk_r[:, 0, :], xf_r[:, 0, sl], start=True, stop=False)
            nc.tensor.matmul(pk, wk_r[:, 1, :], xf_r[:, 1, sl], start=False, stop=True)
            nc.vector.tensor_scalar(
                out=kT[0:32, sl], in0=pk, scalar1=bk2, scalar2=None,
                op0=mybir.AluOpType.add,
            )
            for nt in range(ch * 2, ch * 2 + 2):
                pv = psA.tile([128, C], F32, tag="pj")
                nc.tensor.matmul(
                    pv, xf_r[:, 0, bass.ts(nt, 128)], wv_r[:, 0, :],
                    start=True, stop=False,
                )
                nc.tensor.matmul(
                    pv, xf_r[:, 1, bass.ts(nt, 128)], wv_r[:, 1, :],
                    start=False, stop=True,
                )
                nc.scalar.copy(out=v1T[:, nt, 0:C], in_=pv)
        # fp8 memset is rejected by the ISA checker; convert-copy from f32
        ones32 = const.tile([128, JT, 1], F32)
        nc.vector.memset(ones32, 1.0)
        nc.vector.tensor_copy(out=v1T[:, :, C : C + 1], in_=ones32)

    # residual load: off the critical path, overlaps early attention work
    nc.sync.dma_start(out=x_sb, in_=x_v)

    # xb = x + gamma*bv  (residual with bv folded in; written in place)
    gbv = const.tile([128, CT], F32)
    nc.vector.tensor_scalar(
        out=gbv, in0=bv2, scalar1=gamma, scalar2=None, op0=mybir.AluOpType.mult
    )
    for t in range(CT):
        nc.vector.tensor_scalar(
            out=x_sb[:, t, :], in0=x_sb[:, t, :], scalar1=gbv[:, t : t + 1],
            scalar2=None, op0=mybir.AluOpType.add,
        )

    # ---- attention ----
    def emit_energy(ib, jg):
        # S~^T for 4 key-tiles (K=33 augmented matmuls, uniform PE config);
        # exp in 2 halves so PV can start on the first pair of key-tiles
        # while the second is still in the ACT/DVE pipe
        isl = bass.ds(ib * IB, IB)
        E, E_u8 = E_PAR[ib % 2], E_U8[ib % 2]
        halves = [
            psS.tile([128, JGRP // 2, IB], F32, tag="S", name=f"S_{ib}_{jg}_{h}")
            for h in range(2)
        ]
        for g in range(JGRP):
            jt = jg * JGRP + g
            nc.tensor.matmul(
                halves[g // 2][:, g % 2, :],
                kT[:, bass.ts(jt, 128)],
                qT[:, isl],
                start=True, stop=True,
            )
        for h in range(2):
            t_idx = jg * 2 + h
            jts = jg * JGRP + h * 2
            if t_idx in DVE_TILES:
                nc.vector.tensor_scalar(
                    out=E_u8[:, jts : jts + 2, :],
                    in0=halves[h][:, :, :],
                    scalar1=SCHRAU_A, scalar2=SCHRAU_B,
                    op0=mybir.AluOpType.mult, op1=mybir.AluOpType.add,
                )
            else:
                nc.scalar.activation(
                    out=E[:, jts : jts + 2, :],
                    in_=halves[h][:, :, :],
                    func=mybir.ActivationFunctionType.Exp,
                )

    def emit_pv(ib, jg, po2, base_i_s):
        E = E_PAR[ib % 2]
        for pr in range(JGRP // 2):
            jt = jg * JGRP + pr * 2
            for i_s in (base_i_s, base_i_s + 1):
                nc.tensor.matmul(
                    po2[i_s - base_i_s],
                    E[:, jt : jt + 2, bass.ts(i_s, 128)],
                    v1T[:, jt : jt + 2, :],
                    start=(jt == 0), stop=(jt == JT - 2),
                    perf_mode=mybir.MatmulPerfMode.DoubleRow,
                )

    def emit_epilogue(ib, po2, base_i_s, ot):
        for i_s in (base_i_s, base_i_s + 1):
            po_t = po2[i_s - base_i_s]
            rd = eps.tile([128, 1], F32, tag="rd")
            nc.vector.reciprocal(out=rd, in_=po_t[:, C : C + 1])
            nc.vector.tensor_mul(out=rd, in0=rd, in1=gamma)
            pvn = eps.tile([128, C], BF16, tag="pvn")
            nc.vector.tensor_scalar(
                out=pvn, in0=po_t[:, 0:C], scalar1=rd, scalar2=None,
                op0=mybir.AluOpType.mult,
            )
            pt = psS.tile([128, C], BF16, tag="S")
            nc.tensor.transpose(pt[:, 0:128], pvn[:, 0:128], ident)
            nc.tensor.transpose(pt[:, 128:256], pvn[:, 128:256], ident)
            for t in range(CT):
                nc.vector.tensor_add(
                    out=ot[:, t, bass.ts(i_s, 128)],
                    in0=pt[:, bass.ts(t, 128)],
                    in1=x_sb[:, t, bass.ds(ib * IB + i_s * 128, 128)],
                )

    with (
        tc.tile_pool(name="psS", bufs=3, space="PSUM") as psS,
        tc.tile_pool(name="psO", bufs=2, space="PSUM") as psO,
    ):
        emit_energy(0, 0)
        for ib in range(N_IB):
            # first i-half: PV for i_s 0,1 pipelined with energy+exp of the
            # next group; second i-half afterwards as always-ready PE work
            # that absorbs exp-latency jitter (keeps the HAM clock up)
            po01 = [
                psO.tile([128, C + 1], F32, tag="acc", name=f"po_{ib}_{i_s}")
                for i_s in range(2)
            ]
            for jg in range(N_JG):
                if jg + 1 < N_JG:
                    emit_energy(ib, jg + 1)
                elif ib + 1 < N_IB:
                    emit_energy(ib + 1, 0)
                emit_pv(ib, jg, po01, 0)
            ot = outp.tile([128, CT, IB], F32, tag="ot")
            emit_epilogue(ib, po01, 0, ot)
            po23 = [
                psO.tile([128, C + 1], F32, tag="acc", name=f"po_{ib}_{i_s + 2}")
                for i_s in range(2)
            ]
            for jg in range(N_JG):
                emit_pv(ib, jg, po23, 2)
            emit_epilogue(ib, po23, 2, ot)
            # one batched store per (i-block, channel-tile): each dma_start
            # costs ~660ns of sync-queue issue time
            for t in range(CT):
                nc.sync.dma_start(
                    out=out_v[:, t, bass.ts(ib, IB)], in_=ot[:, t, :]
                )
        if DBG:
            nc.sync.dma_start(out=qTd_e[:, :], in_=qT.bitcast(mybir.dt.uint16))
            nc.sync.dma_start(out=kTd_e[:, :], in_=kT.bitcast(mybir.dt.uint16))
            nc.sync.dma_start(out=v1d_e[:, :], in_=v1T.bitcast(U8).rearrange("p a b -> p (a b)"))
            nc.sync.dma_start(out=E0d_e[:, :], in_=E0.bitcast(U8).rearrange("p a b -> p (a b)"))


_CACHE = {}


def _build():
    if "nc" not in _CACHE:
        nc = bass.Bass()
        from contextlib import ExitStack
        with PatchedTileContext(nc) as tc, ExitStack() as ctx:
            _attention_body(nc, tc, ctx)
        _CACHE["nc"] = nc
    return _CACHE["nc"]


def _host_shift(x, wq, bq, wk, bk):
    """Per-query exp shift m (one [1, N] row per batch). Only numerics
    depend on it (it cancels in softmax): m must sit within about
    [rowmax - 10, rowmax + 8] of each query's true row max so that
    exp(S - m) fits e5m2's dynamic range. Built from cheap host features:
    sample maxes over the 256 largest-|k| columns and a 16-strided comb,
    plus a |q|-norm linear fit."""
    xf = x.reshape(B, C, N).astype(np.float32)
    q = np.einsum('oc,bcn->bno', wq.astype(np.float32), xf) + bq.astype(np.float32)
    k = np.einsum('oc,bcn->bon', wk.astype(np.float32), xf) \
        + bk.astype(np.float32)[None, :, None]
    kn = np.linalg.norm(k, axis=1)
    idx = np.argpartition(-kn, 256, axis=-1)[:, :256]
    ksel = np.take_along_axis(k, idx[:, None, :], axis=2)
    topk = np.einsum('bno,bom->bnm', q, ksel).max(-1)
    smax16 = np.einsum('bno,bom->bnm', q, k[:, :, ::16]).max(-1)
    qn = np.linalg.norm(q, axis=-1)
    fit = 3.916 * qn - 0.737
    return np.maximum.reduce([topk + 0.5, smax16 + 0.5, fit + 2.0])


def _aug_rows(row0):
    a = np.zeros((32, N), np.uint16)
    a[0] = np.frombuffer(
        row0.astype(np.float32).astype(ml_dtypes.bfloat16).tobytes(),
        dtype=np.uint16)
    return np.ascontiguousarray(a)


def _prep_in_maps(x, wq, bq, wk, bk, wv, bv, gamma):
    asc = np.ascontiguousarray
    wqt2 = asc(wq.T.astype(np.float32))                     # [C, 32]
    wkt2 = asc(wk.T.astype(np.float32))
    wvt = asc(wv.T.astype(np.float32))                      # [C, C]
    bq2 = asc(bq[:, None].astype(np.float32))
    bk2 = asc(bk[:, None].astype(np.float32))
    bv2 = asc(bv.reshape(CT, 128).T.astype(np.float32))     # [128, CT]
    g128 = np.full((128, 1), np.float32(gamma[0]), dtype=np.float32)
    m = _host_shift(x, wq, bq, wk, bk)                      # [B, N]
    maps = []
    for b in range(B):
        maps.append({
            "x": asc(x[b].reshape(C, N).astype(np.float32)),
            "wqt2": wqt2, "wkt2": wkt2, "wvt": wvt,
            "bq2": bq2, "bk2": bk2, "bv2": bv2, "gamma128": g128,
            "qaug": _aug_rows(-m[b]),
            "kaug": _aug_rows(np.ones(N, np.float32)),
        })
    return maps


def _run(inputs, trace=False):
    nc = _build()
    in_maps = _prep_in_maps(**{k: np.asarray(v) for k, v in inputs.items()})
    res = run_bass_kernel_spmd(nc, in_maps, list(range(NCORES)), trace=trace)
    out = np.stack([res.results[b]["out"].reshape(C, H, W) for b in range(B)])
    return out.astype(np.float32), res


def kernel(**inputs):
    out, _ = _run(inputs, trace=False)
    return out
